# revision 20
# baseline (speedup 1.0000x reference)
"""Two-layer GAT on 8 Trainium2 NeuronCores (Bass/Tile).

Sharding: nodes split 12500/core; edges partitioned by destination core.
Per core, per layer:
  GEMM phase computes [h | s_src | s_dst | skip] columns in one pass
  (attention vectors folded as W@a columns; biases folded via a ones-row).
  Rows are staged partition-major ([128, nt*AGW] bf16) so the AllGather
  input DMA is 128 large descriptors; ONE AllGather per layer lands the
  full 8-core table in a DRAM tensor indexed by
  row = core*nlocp + p*nt + t.
  Edge phase: chunks of 128 edges grouped (~GC chunks); per group:
    - one indirect-DMA gather per chunk (128 rows; the instruction
      supports only one index per partition)
    - ONE DMA of the group's transposed one-hots (partition-major DRAM
      layout -> 128 big descriptors)
    - per-chunk esd matmul (s_dst expansion) into group PSUM columns,
      ONE PSUM->SBUF copy per group
    - p = exp(leaky_relu(ssrc+sdst)) batched: max(exp(u), exp(0.2u))
    - per chunk: oh2 = (iota == dstloc)*p on DVE; matmul-accumulate
      psum[128 nodes, AGW]; column 129 accumulates the denominator.
  Layer 1 skip path (x@lin1_W + b) is written straight into the h1relu
  SBUF buffer during GEMM1 (no DRAM bounce). Layer 2 aggregates in
  128-dim h1relu space (aggregation commutes with @W2):
  out = (agg2/denom)@W2 + h1relu@lin2_W + biases.
"""
import sys

if '/opt/trn_rl_repo' not in sys.path:
    sys.path.insert(0, '/opt/trn_rl_repo')

import numpy as np

P = 128
NEG = 0.2
AGW = 132          # staged row: [feat(128) | ssrc | one | pad pad]
EPS = 1e-16


def _cfg(N, E, DIN, D1, D2, ncore, group_chunks=28):
    assert N % ncore == 0
    nloc = N // ncore
    nt = -(-nloc // P)
    return dict(N=N, E=E, DIN=DIN, D1=D1, D2=D2, NCORE=ncore,
                NLOC=nloc, NT=nt, NLOCP=nt * P,
                K1=-(-(DIN + 1) // P), GC=group_chunks)


def _quarters(nt, nq=4):
    bounds = [int(round(i * nt / nq)) for i in range(nq + 1)]
    return [(bounds[i], bounds[i + 1]) for i in range(nq)]


def _pack_tiles(deg, nt):
    """Balance in-degree over tiles: tiles 0..nt-2 get 128 nodes each via
    a snake-deal of degree-sorted nodes (loads land ~equal, below 512);
    the last tile takes the highest-degree nodes plus the pad slots.
    Returns newpos[orig_local] in [0, nt*128)."""
    nloc = len(deg)
    nfull = nt - 1
    ntop = nloc - nfull * P
    order = np.argsort(-deg, kind='stable')
    top, rest = order[:ntop], order[ntop:]
    rr = rest.reshape(P, nfull).copy()
    rr[1::2] = rr[1::2, ::-1]
    newpos = np.empty(nloc, np.int64)
    newpos[rr.reshape(-1)] = (np.tile(np.arange(nfull), P) * P
                              + np.repeat(np.arange(P), nfull))
    newpos[top] = nfull * P + np.arange(ntop)
    return newpos


def preprocess(x, edge_index, cfg):
    """Host-side index prep.

    Returns (aux_per_core, cpt_t) where cpt_t[t] = chunks for tile t
    (max over cores, so the SPMD program structure is shared).
    aux arrays are laid out [128, nch_total] in global chunk order.
    ohT is the transposed one-hot table, partition-major:
    ohT[m, q*128+e] = 1 if edge e of chunk q has dst-local == m.
    Nodes are re-packed into tiles per-core ('perm') so per-tile edge
    counts are balanced (fewer padded chunks)."""
    import ml_dtypes
    ncore, nloc, nt = cfg['NCORE'], cfg['NLOC'], cfg['NT']
    nlocp = cfg['NLOCP']
    src = np.asarray(edge_index[0], np.int64)
    dst = np.asarray(edge_index[1], np.int64)

    perms = []
    for c in range(ncore):
        d = dst[(dst // nloc) == c] - c * nloc
        deg = np.bincount(d, minlength=nloc)
        perms.append(_pack_tiles(deg, nt))

    per_core = []
    counts_all = np.zeros((ncore, nt), np.int64)
    for c in range(ncore):
        m = (dst // nloc) == c
        s = src[m]
        d = perms[c][dst[m] - c * nloc]   # permuted local position
        t = d // P
        order = np.argsort(t, kind='stable')
        s, d, t = s[order], d[order], t[order]
        counts = np.bincount(t, minlength=nt)
        counts_all[c] = counts
        starts = np.zeros(nt + 1, np.int64)
        np.cumsum(counts, out=starts[1:])
        per_core.append((s, d, starts))

    cpt_t = np.maximum(1, -(-counts_all.max(axis=0) // P))  # [nt]
    nch = int(cpt_t.sum())
    chunk_t0 = np.zeros(nt + 1, np.int64)
    np.cumsum(cpt_t, out=chunk_t0[1:])

    # permuted source position for every global node, per owning core
    srcpos_all = np.concatenate(
        [perms[co] for co in range(ncore)])  # [N] position within core

    quarts = _quarters(nt)
    qrows = np.array([b - a for a, b in quarts])
    qa_of = np.zeros(nt, np.int64)
    qid_of = np.zeros(nt, np.int64)
    for qi, (a, b) in enumerate(quarts):
        qid_of[a:b] = qi
        qa_of[a:b] = a
    qbase = np.zeros(len(quarts), np.int64)
    acc = 0
    for qi, (a, b) in enumerate(quarts):
        qbase[qi] = acc
        acc += ncore * (b - a) * P

    aux = []
    for c in range(ncore):
        s, d, starts = per_core[c]
        s_co = s // nloc
        s_pos = srcpos_all[s]
        src_gidx = np.zeros((nch, P), np.int32)
        dstloc_f = np.full((nch, P), -1.0, np.float32)
        for t in range(nt):
            e0, e1 = starts[t], starts[t + 1]
            n = e1 - e0
            q0 = chunk_t0[t]
            buf_s = np.zeros(int(cpt_t[t]) * P, np.int32)
            buf_d = np.full(int(cpt_t[t]) * P, -1.0, np.float32)
            sp_ = s_pos[e0:e1]
            st = sp_ // P
            qi = qid_of[st]
            buf_s[:n] = (qbase[qi] + s_co[e0:e1] * P * qrows[qi]
                         + (sp_ % P) * qrows[qi]
                         + (st - qa_of[st])).astype(np.int32)
            buf_d[:n] = (d[e0:e1] % P).astype(np.float32)
            src_gidx[q0:q0 + cpt_t[t]] = buf_s.reshape(-1, P)
            dstloc_f[q0:q0 + cpt_t[t]] = buf_d.reshape(-1, P)
        flat_d = dstloc_f.reshape(1, nch * P)
        ohT = (np.arange(P, dtype=np.float32)[:, None] == flat_d)
        ohT = ohT.astype(ml_dtypes.bfloat16)
        aux.append(dict(src_gidx=np.ascontiguousarray(src_gidx.T),
                        dstloc_f=np.ascontiguousarray(dstloc_f.T),
                        ohT=np.ascontiguousarray(ohT),
                        perm=perms[c]))
    return aux, cpt_t


def make_xt_tiles(x, cfg, c, perm):
    """Per-core transposed, padded x, partition-major: [128, nt*k1*128]
    bf16 with xt[p, t*k1*128 + k*128 + cc] = xl[t*128+cc, k*128+p].
    Rows are placed at their packed positions (perm)."""
    import ml_dtypes
    nloc, nt, k1, din = cfg['NLOC'], cfg['NT'], cfg['K1'], cfg['DIN']
    xl = np.zeros((nt * P, k1 * P), np.float32)
    xl[perm, :din] = x[c * nloc:(c + 1) * nloc]
    xl[perm, din] = 1.0  # ones column feeds folded biases
    # [nt, P(node), k1, P(feat)] -> [P(feat), nt, k1, P(node)]
    blocks = xl.reshape(nt, P, k1, P).transpose(3, 0, 2, 1)
    return np.ascontiguousarray(
        blocks.reshape(P, nt * k1 * P).astype(ml_dtypes.bfloat16))


def build(cfg, cpt_t, weights):
    import concourse.bass as bass
    import concourse.bacc as bacc
    import concourse.mybir as mybir
    import concourse.tile as tile

    DIN, D1, D2 = cfg['DIN'], cfg['D1'], cfg['D2']
    ncore, nloc, nt, nlocp, k1 = (cfg['NCORE'], cfg['NLOC'], cfg['NT'],
                                  cfg['NLOCP'], cfg['K1'])
    gc_target = cfg['GC']
    nch = int(cpt_t.sum())
    chunk_t0 = np.zeros(nt + 1, np.int64)
    np.cumsum(cpt_t, out=chunk_t0[1:])
    assert D1 == P

    # group tiles so each group has ~gc_target chunks
    groups = []
    t0 = 0
    while t0 < nt:
        t1 = t0 + 1
        while t1 < nt and chunk_t0[t1 + 1] - chunk_t0[t0] <= gc_target:
            t1 += 1
        groups.append((t0, t1))
        t0 = t1

    W1, a_src1, a_dst1, b1 = (weights[k] for k in ('W1', 'a_src1', 'a_dst1', 'b1'))
    lin1_W, lin1_b = weights['lin1_W'], weights['lin1_b']
    W2, a_src2, a_dst2, b2 = (weights[k] for k in ('W2', 'a_src2', 'a_dst2', 'b2'))
    lin2_W, lin2_b = weights['lin2_W'], weights['lin2_b']

    c1 = 2 * D1 + 2
    w1aug = np.zeros((k1 * P, c1), np.float32)
    w1aug[:DIN, 0:D1] = W1
    w1aug[:DIN, D1] = W1 @ a_src1
    w1aug[:DIN, D1 + 1] = W1 @ a_dst1
    w1aug[:DIN, D1 + 2:] = lin1_W
    w1aug[DIN, D1 + 2:] = b1 + lin1_b
    w2a = np.stack([W2 @ a_src2, W2 @ a_dst2], axis=1).astype(np.float32)
    bias2 = np.tile((b2 + lin2_b)[None, :], (P, 1)).astype(np.float32)
    iota_np = np.tile(np.arange(P, dtype=np.float32), (P, 1))
    ident_np = np.eye(P, dtype=np.float32)

    nc = bacc.Bacc("TRN2", target_bir_lowering=False, debug=False,
                   num_devices=ncore)
    f32, bf16, i32 = mybir.dt.float32, mybir.dt.bfloat16, mybir.dt.int32

    xt_t = nc.dram_tensor("xt_tiles", [P, nt * k1 * P], bf16, kind="ExternalInput")
    idx_t = nc.dram_tensor("src_gidx", [P, nch], i32, kind="ExternalInput")
    dstf_t = nc.dram_tensor("dstloc_f", [P, nch], f32, kind="ExternalInput")
    ohT_t = nc.dram_tensor("ohT", [P, nch * P], bf16, kind="ExternalInput")
    out_t = nc.dram_tensor("out", [nlocp, D2], f32, kind="ExternalOutput")

    import ml_dtypes
    w1aug_c = nc.inline_tensor(w1aug.astype(ml_dtypes.bfloat16), name="w1aug")
    w2a_c = nc.inline_tensor(w2a.astype(ml_dtypes.bfloat16), name="w2a")
    w2_c = nc.inline_tensor(W2.astype(ml_dtypes.bfloat16), name="w2c")
    lin2_c = nc.inline_tensor(lin2_W.astype(ml_dtypes.bfloat16), name="lin2c")
    bias2_c = nc.inline_tensor(bias2, name="bias2c")
    iota_c = nc.inline_tensor(iota_np, name="iotac")
    ident_c = nc.inline_tensor(ident_np, name="identc")

    rg = [list(range(ncore))]
    XTW = k1 * P  # xt columns per tile

    with tile.TileContext(nc) as tc:
        with (
            tc.tile_pool(name="persist", bufs=1) as pp,
            tc.tile_pool(name="work", bufs=2) as wp,
            tc.tile_pool(name="moh", bufs=6) as mp,
            tc.tile_pool(name="gath", bufs=3) as gp,
            tc.tile_pool(name="psum", bufs=2, space="PSUM") as pep,
            tc.tile_pool(name="dram", bufs=1, space="DRAM") as dp,
        ):
            iota_sb = pp.tile([P, P], f32)
            nc.sync.dma_start(out=iota_sb[:], in_=iota_c.ap())
            ident_sb = pp.tile([P, P], f32)
            nc.sync.dma_start(out=ident_sb[:], in_=ident_c.ap())
            w1_sb = pp.tile([P, k1, c1], bf16)
            nc.sync.dma_start(
                out=w1_sb[:],
                in_=w1aug_c.ap().rearrange("(k r) c -> r k c", k=k1))
            w2a_sb = pp.tile([P, 2], bf16)
            nc.sync.dma_start(out=w2a_sb[:], in_=w2a_c.ap())
            w2_sb = pp.tile([P, D2], bf16)
            nc.sync.dma_start(out=w2_sb[:], in_=w2_c.ap())
            lin2_sb = pp.tile([P, D2], bf16)
            nc.sync.dma_start(out=lin2_sb[:], in_=lin2_c.ap())
            bias2_sb = pp.tile([P, D2], f32)
            nc.sync.dma_start(out=bias2_sb[:], in_=bias2_c.ap())
            idx_sb = pp.tile([P, nch], i32)
            nc.sync.dma_start(out=idx_sb[:], in_=idx_t[:, :])
            dstf_sb = pp.tile([P, nch], f32)
            nc.sync.dma_start(out=dstf_sb[:], in_=dstf_t[:, :])

            h1relu_sb = pp.tile([P, nt * P], f32)
            h1T_sb = pp.tile([P, nt * P], bf16)
            sdst1_sb = pp.tile([P, nt], bf16)
            sdst2_sb = pp.tile([P, nt], bf16)

            quarts = _quarters(nt)
            qbase = []
            acc = 0
            for (a, b) in quarts:
                qbase.append(acc)
                acc += ncore * (b - a) * P
            ag1_inQ = [dp.tile([P, (b - a) * AGW], bf16, name=f"ag1i{i}")
                       for i, (a, b) in enumerate(quarts)]
            ag2_inQ = [dp.tile([P, (b - a) * AGW], bf16, name=f"ag2i{i}")
                       for i, (a, b) in enumerate(quarts)]
            ag1_out = dp.tile([ncore * nlocp, AGW], bf16, name="ag1o")
            ag2_out = dp.tile([ncore * nlocp, AGW], bf16, name="ag2o")

            def new_stage():
                st = wp.tile([P, nt * AGW], bf16, tag="stage", bufs=1)
                nc.vector.memset(
                    st[:].rearrange("p (t w) -> p t w", t=nt)[:, :, D1 + 1:D1 + 2],
                    1.0)
                nc.vector.memset(
                    st[:].rearrange("p (t w) -> p t w", t=nt)[:, :, D1 + 2:AGW],
                    0.0)
                return st

            def emit_ag_quarter(stage, ag_inQ, ag_out, qi):
                a, b = quarts[qi]
                rows = ncore * (b - a) * P
                nc.sync.dma_start(out=ag_inQ[qi][:, :],
                                  in_=stage[:, a * AGW:b * AGW])
                nc.gpsimd.collective_compute(
                    "AllGather", mybir.AluOpType.bypass, replica_groups=rg,
                    ins=[ag_inQ[qi][:].opt()],
                    outs=[ag_out[qbase[qi]:qbase[qi] + rows, :].opt()])

            qend_of = {}
            for qi, (a, b) in enumerate(quarts):
                qend_of[b - 1] = qi

            # ================= Phase 1: GEMM1 ==============================
            stage1 = new_stage()
            XTB = 4  # tiles per xt load
            for tb in range(0, nt, XTB):
                tbn = min(XTB, nt - tb)
                xt_sb = wp.tile([P, XTB * XTW], bf16, tag="xt", bufs=2)
                nc.sync.dma_start(
                    out=xt_sb[:, 0:tbn * XTW],
                    in_=xt_t[:, tb * XTW:(tb + tbn) * XTW])
                for ti in range(tbn):
                    t = tb + ti
                    ps1 = pep.tile([P, c1], f32, tag="g1", space="PSUM")
                    for k in range(k1):
                        nc.tensor.matmul(
                            out=ps1[:],
                            lhsT=xt_sb[:, ti * XTW + k * P: ti * XTW + (k + 1) * P],
                            rhs=w1_sb[:, k, :],
                            start=(k == 0), stop=(k == k1 - 1))
                    nc.vector.tensor_copy(
                        out=stage1[:, t * AGW:t * AGW + D1 + 1],
                        in_=ps1[:, 0:D1 + 1])
                    nc.vector.tensor_copy(out=sdst1_sb[:, t:t + 1],
                                          in_=ps1[:, D1 + 1:D1 + 2])
                    nc.scalar.copy(out=h1relu_sb[:, t * P:(t + 1) * P],
                                   in_=ps1[:, D1 + 2:c1])
                    if t in qend_of:
                        emit_ag_quarter(stage1, ag1_inQ, ag1_out, qend_of[t])

            # ============ Edge phase (shared between both layers) ==========
            def edge_phase(ag_out, sdst_sb, evict_tile):
                for (t0, t1) in groups:
                    q0, q1 = int(chunk_t0[t0]), int(chunk_t0[t1])
                    gcn = q1 - q0
                    g_grp = gp.tile([P, gcn * AGW], bf16, tag="g")
                    for q in range(q0, q1):
                        j = q - q0
                        nc.gpsimd.indirect_dma_start(
                            out=g_grp[:, j * AGW:(j + 1) * AGW],
                            out_offset=None,
                            in_=ag_out[:],
                            in_offset=bass.IndirectOffsetOnAxis(
                                ap=idx_sb[:, q:q + 1], axis=0))
                    oht_grp = gp.tile([P, gcn * P], bf16, tag="oh")
                    nc.sync.dma_start(out=oht_grp[:],
                                      in_=ohT_t[:, q0 * P:q1 * P])
                    esd_ps = pep.tile([P, gcn], f32, tag="esd", space="PSUM")
                    for t in range(t0, t1):
                        qa, qb = int(chunk_t0[t]), int(chunk_t0[t + 1])
                        for q in range(qa, qb):
                            j = q - q0
                            nc.tensor.matmul(
                                out=esd_ps[:, j:j + 1],
                                lhsT=oht_grp[:, j * P:(j + 1) * P],
                                rhs=sdst_sb[:, t:t + 1],
                                start=True, stop=True)
                    esd = wp.tile([P, gcn], f32, tag="esd")
                    nc.scalar.copy(out=esd[:], in_=esd_ps[:])
                    # batched scores for the group
                    u = wp.tile([P, gcn], f32, tag="u")
                    nc.vector.tensor_tensor(
                        out=u[:], in0=esd[:],
                        in1=g_grp[:].rearrange("p (k e) -> p k e", k=gcn)[:, :, D1],
                        op=mybir.AluOpType.add)
                    e1 = wp.tile([P, gcn], f32, tag="e1")
                    nc.scalar.activation(out=e1[:], in_=u[:],
                                         func=mybir.ActivationFunctionType.Exp)
                    e2 = wp.tile([P, gcn], f32, tag="e2")
                    nc.scalar.activation(out=e2[:], in_=u[:],
                                         func=mybir.ActivationFunctionType.Exp,
                                         scale=NEG)
                    pt = wp.tile([P, gcn], f32, tag="p")
                    nc.vector.tensor_tensor(out=pt[:], in0=e1[:], in1=e2[:],
                                            op=mybir.AluOpType.max)
                    # one-hot scale + matmul accumulate, per tile
                    for t in range(t0, t1):
                        pse = pep.tile([P, AGW], f32, tag="edge", space="PSUM")
                        qa, qb = int(chunk_t0[t]), int(chunk_t0[t + 1])
                        for q in range(qa, qb):
                            j = q - q0
                            oh2 = mp.tile([P, P], bf16, tag="oh2")
                            nc.vector.tensor_scalar(
                                out=oh2[:], in0=iota_sb[:],
                                scalar1=dstf_sb[:, q:q + 1],
                                scalar2=pt[:, j:j + 1],
                                op0=mybir.AluOpType.is_equal,
                                op1=mybir.AluOpType.mult)
                            nc.tensor.matmul(
                                out=pse[:], lhsT=oh2[:],
                                rhs=g_grp[:, j * AGW:(j + 1) * AGW],
                                start=(q == qa), stop=(q == qb - 1))
                        evict_tile(t, pse)

            # ---- layer-1 eviction: h1relu + GEMM2a + ag2 assembly ---------
            stage2 = new_stage()

            def evict1(t, pse):
                den = wp.tile([P, 1], f32, tag="den")
                nc.vector.tensor_scalar(out=den[:], in0=pse[:, D1 + 1:D1 + 2],
                                        scalar1=EPS, scalar2=None,
                                        op0=mybir.AluOpType.add)
                rec = wp.tile([P, 1], f32, tag="rec")
                nc.vector.reciprocal(out=rec[:], in_=den[:])
                tmp = wp.tile([P, P], f32, tag="ev1a")
                nc.vector.tensor_scalar(out=tmp[:], in0=pse[:, 0:D1],
                                        scalar1=rec[:, 0:1], scalar2=None,
                                        op0=mybir.AluOpType.mult)
                tmp2 = wp.tile([P, P], f32, tag="ev1b")
                nc.vector.tensor_tensor(out=tmp2[:], in0=tmp[:],
                                        in1=h1relu_sb[:, t * P:(t + 1) * P],
                                        op=mybir.AluOpType.add)
                nc.scalar.activation(out=h1relu_sb[:, t * P:(t + 1) * P],
                                     in_=tmp2[:],
                                     func=mybir.ActivationFunctionType.Relu)
                tp = pep.tile([P, P], f32, tag="t", space="PSUM", bufs=1)
                nc.tensor.transpose(out=tp[:],
                                    in_=h1relu_sb[:, t * P:(t + 1) * P],
                                    identity=ident_sb[:])
                nc.scalar.copy(out=h1T_sb[:, t * P:(t + 1) * P], in_=tp[:])
                sc = pep.tile([P, 2], f32, tag="g1", space="PSUM")
                nc.tensor.matmul(out=sc[:],
                                 lhsT=h1T_sb[:, t * P:(t + 1) * P],
                                 rhs=w2a_sb[:], start=True, stop=True)
                nc.vector.tensor_copy(out=stage2[:, t * AGW:t * AGW + D1],
                                      in_=h1relu_sb[:, t * P:(t + 1) * P])
                nc.vector.tensor_copy(
                    out=stage2[:, t * AGW + D1:t * AGW + D1 + 1],
                    in_=sc[:, 0:1])
                nc.vector.tensor_copy(out=sdst2_sb[:, t:t + 1], in_=sc[:, 1:2])
                if t in qend_of:
                    emit_ag_quarter(stage2, ag2_inQ, ag2_out, qend_of[t])

            edge_phase(ag1_out, sdst1_sb, evict1)

            # ---- layer-2 eviction: out = (agg2/den)@W2 + h1relu@lin2 + b --
            def evict2(t, pse):
                den = wp.tile([P, 1], f32, tag="den")
                nc.vector.tensor_scalar(out=den[:], in0=pse[:, D1 + 1:D1 + 2],
                                        scalar1=EPS, scalar2=None,
                                        op0=mybir.AluOpType.add)
                rec = wp.tile([P, 1], f32, tag="rec")
                nc.vector.reciprocal(out=rec[:], in_=den[:])
                a2n = wp.tile([P, P], f32, tag="a2n")
                nc.vector.tensor_scalar(out=a2n[:], in0=pse[:, 0:D1],
                                        scalar1=rec[:, 0:1], scalar2=None,
                                        op0=mybir.AluOpType.mult)
                tp1 = pep.tile([P, P], f32, tag="t", space="PSUM", bufs=1)
                nc.tensor.transpose(out=tp1[:], in_=a2n[:], identity=ident_sb[:])
                a2T = wp.tile([P, P], bf16, tag="a2T")
                nc.scalar.copy(out=a2T[:], in_=tp1[:])
                po = pep.tile([P, D2], f32, tag="out", space="PSUM", bufs=1)
                nc.tensor.matmul(out=po[:], lhsT=a2T[:], rhs=w2_sb[:],
                                 start=True, stop=False)
                nc.tensor.matmul(out=po[:],
                                 lhsT=h1T_sb[:, t * P:(t + 1) * P],
                                 rhs=lin2_sb[:], start=False, stop=True)
                osb = wp.tile([P, D2], f32, tag="osb", bufs=3)
                nc.vector.tensor_tensor(out=osb[:], in0=po[:], in1=bias2_sb[:],
                                        op=mybir.AluOpType.add)
                nc.sync.dma_start(out=out_t[t * P:(t + 1) * P, :],
                                  in_=osb[:])

            edge_phase(ag2_out, sdst2_sb, evict2)

    nc.compile()
    return nc


def _run(nc, in_maps, ncore, trace=False, **kw):
    from concourse import bass_utils
    return bass_utils.run_bass_kernel_spmd(
        nc, in_maps, core_ids=list(range(ncore)), trace=trace, **kw)


_CACHE = {}


def kernel(**inputs):
    x = np.asarray(inputs['x'], np.float32)
    edge_index = np.asarray(inputs['edge_index'], np.int32)
    cfg = _cfg(N=100000, E=400000, DIN=300, D1=128, D2=512, ncore=8)
    weights = {k: np.asarray(v, np.float32) for k, v in inputs.items()
               if k not in ('x', 'edge_index')}

    aux, cpt_t = preprocess(x, edge_index, cfg)
    key = ('nn_gat', int(cpt_t.sum()))
    if key not in _CACHE:
        _CACHE[key] = build(cfg, cpt_t, weights)
    nc = _CACHE[key]

    in_maps = []
    for c in range(cfg['NCORE']):
        in_maps.append(dict(
            xt_tiles=make_xt_tiles(x, cfg, c, aux[c]['perm']),
            src_gidx=aux[c]['src_gidx'],
            dstloc_f=aux[c]['dstloc_f'],
            ohT=aux[c]['ohT'],
        ))
    res = _run(nc, in_maps, cfg['NCORE'])
    out = np.concatenate(
        [res.results[c]['out'][aux[c]['perm']] for c in range(cfg['NCORE'])],
        axis=0)
    return out
